# revision 30
# baseline (speedup 1.0000x reference)
"""SPP-net detection head (pooling + FC stack + dual softmax) on 8 TRN2 cores.

Strategy (tensor-parallel, all compute on device, ~560 us HW exec):
  - The per-ROI 7x7/s7 max-pool of a 14x14 crop == 4 point-lookups into a
    dense pooled map N[c, y*64+x] = max of the 7x7 window at (y, x), computed
    once via two 3-op max-trees (DVE). Channel-sharded: 64 channels/core,
    then PE-transposed to NT[pos, ch] in DRAM.
  - ROI gather: native indirect-DMA row-gathers straight from NT, one
    [128 ROIs, 64 ch] gather per (ROI-block, corner) with the corner offset
    folded into the device-computed index column (64 gathers total, no
    extended-ISA library, no table build).
  - Pipeline over 2 ROI chunks of 1024, software-pipelined emission so each
    engine FIFO overlaps: gathers+transposes+AGf triggers first, then
    FC6+AGh per chunk, then FC7+FC8+AR per chunk.
    FC6 fp32r (TP by output, folded w6eff = w6[:, :2048]+w6[:, 2048:] via
    DVE add), h1 wire + FC7 in bf16, FC8 fp32r with both heads fused into a
    padded [64, r] tile (partitions 0:21 / 32:53), one AllReduce per chunk.
  - A tiny dummy AllGather at kernel start absorbs the ncfw first-call cost.
  - Softmax-over-ROIs in the transposed [21, 2048] layout (free-dim
    reduction); PE-transpose to [r, 21]; class softmax via broadcast-AP
    vector ops; score via a ones-matmul accumulated in PSUM.

Host side does layout-only work: slicing, transposition, and a feature
permutation matching the corner-major gather order (f_local = k*64 + c).
"""
import os
import sys

import numpy as np

for _p in ("/opt/trn_rl_repo", "/root/.axon_site/_ro/trn_rl_repo"):
    if os.path.isdir(_p) and _p not in sys.path:
        sys.path.append(_p)

import concourse.bacc as bacc
import concourse.bass as bass
import concourse.mybir as mybir
import concourse.tile as tile
from concourse.bass_utils import run_bass_kernel_spmd

F32 = mybir.dt.float32
F32R = mybir.dt.float32r
BF16 = mybir.dt.bfloat16
I32 = mybir.dt.int32
AF = mybir.ActivationFunctionType
ALU = mybir.AluOpType
AX = mybir.AxisListType

NCORES = 8
CORE_IDS = list(range(NCORES))
R = 2048          # ROIs
NCH = 2           # ROI chunks in the pipeline
RC = R // NCH     # 512 ROIs per chunk
C_LOC = 64        # channels per core
F_LOC = 256       # features per core (4 corners x 64 ch)
NCLS = 21
NH = 64           # padded dual-head dim (xc at 0:21, xd at 32:53)
POS = 3712        # pooled-map positions (58 rows x 64 cols, x<=57 valid)
CKS = (0, 7, 448, 455)   # corner offsets: 7*64*(k//2) + 7*(k%2)

TRACE = False
LAST_EXEC_NS = None

_CACHE = {}


def _build():
    nc = bacc.Bacc("TRN2", target_bir_lowering=False, debug=False,
                   num_devices=NCORES)

    # ---- I/O ----
    xc_d = nc.dram_tensor("xc", [C_LOC, 4096], F32, kind="ExternalInput")
    ssw_d = nc.dram_tensor("ssw", [R, 4], I32, kind="ExternalInput")
    w6a_d = nc.dram_tensor("w6a", [2048, 512], F32, kind="ExternalInput")
    w6b_d = nc.dram_tensor("w6b", [2048, 512], F32, kind="ExternalInput")
    w7s_d = nc.dram_tensor("w7s", [4096, 512], F32, kind="ExternalInput")
    w8s_d = nc.dram_tensor("w8s", [512, NH], F32, kind="ExternalInput")
    b6s_d = nc.dram_tensor("b6s", [512], F32, kind="ExternalInput")
    b7s_d = nc.dram_tensor("b7s", [512], F32, kind="ExternalInput")
    b8_d = nc.dram_tensor("b8", [NH], F32, kind="ExternalInput")
    ident_d = nc.dram_tensor("ident", [128, 128], F32, kind="ExternalInput")

    dm_d = nc.dram_tensor("dm", [R, NCLS], F32, kind="ExternalOutput")
    dr_d = nc.dram_tensor("dr", [R, NCLS], F32, kind="ExternalOutput")
    score_d = nc.dram_tensor("score", [1, NCLS], F32, kind="ExternalOutput")

    with tile.TileContext(nc) as tc:
        with (
            tc.tile_pool(name="wp", bufs=1) as wp,        # persistent weights
            tc.tile_pool(name="sb", bufs=1) as sb,        # misc working tiles
            tc.tile_pool(name="st", bufs=2) as st,        # streaming tiles
            tc.tile_pool(name="ps", bufs=8, space="PSUM") as ps,
            tc.tile_pool(name="dram", bufs=1, space="DRAM") as dram,
        ):
            # ================= ROI indices + small loads ============
            sswf = ssw_d[:].rearrange("r c -> (r c)")
            y0_t = sb.tile([128, 16], I32)
            nc.sync.dma_start(y0_t[:], sswf.rearrange("(j p c) -> p j c",
                                                      p=128, c=4)[:, :, 0])
            x0_t = sb.tile([128, 16], I32)
            nc.sync.dma_start(x0_t[:], sswf.rearrange("(j p c) -> p j c",
                                                      p=128, c=4)[:, :, 1])
            id_t = wp.tile([128, 128], F32)
            nc.sync.dma_start(id_t[:], ident_d[:])
            y0f = sb.tile([128, 16], F32)
            nc.vector.tensor_copy(y0f[:], y0_t[:])
            x0f = sb.tile([128, 16], F32)
            nc.vector.tensor_copy(x0f[:], x0_t[:])
            basef = sb.tile([128, 16], F32)
            nc.vector.scalar_tensor_tensor(basef[:], y0f[:], 64.0, x0f[:],
                                           op0=ALU.mult, op1=ALU.add)
            idx4f = sb.tile([128, 64], F32)
            for k, ck in enumerate(CKS):
                nc.vector.tensor_scalar_add(idx4f[:, 16 * k:16 * k + 16],
                                            basef[:], float(ck))
            idx_t = sb.tile([128, 64], I32)
            nc.vector.tensor_copy(idx_t[:], idx4f[:])

            # x on the scalar HWDGE queue so pooling starts immediately;
            # big weight loads go on sync behind the tiny index loads.
            w6e_t = wp.tile([128, 16 * 512], F32R)
            nc.sync.dma_start(
                w6e_t[:].rearrange("p (q e) -> p q e", e=512),
                w6a_d[:].bitcast(F32R).rearrange("(q p) e -> p q e", p=128))
            w8_t = wp.tile([128, 4 * NH], F32R)
            nc.scalar.dma_start(
                w8_t[:].rearrange("p (q c) -> p q c", c=NH),
                w8s_d[:].bitcast(F32R).rearrange("(q p) c -> p q c", p=128))
            b6_t = wp.tile([128, 4], F32)
            nc.scalar.dma_start(b6_t[:], b6s_d[:].rearrange("(m p) -> p m", p=128))
            b7_t = wp.tile([128, 4], F32)
            nc.scalar.dma_start(b7_t[:], b7s_d[:].rearrange("(m p) -> p m", p=128))
            b8_t = wp.tile([NH, 1], F32)
            nc.scalar.dma_start(b8_t[:], b8_d[:].rearrange("(p o) -> p o", o=1))

            # tiny dummy collective to absorb ncfw first-call warmup early
            wu_in = dram.tile([16, 16], F32)
            wu_out = dram.tile([128, 16], F32, addr_space="Shared")
            nc.sync.dma_start(wu_in[:], id_t[0:16, 0:16])
            nc.gpsimd.collective_compute(
                "AllGather", ALU.bypass, replica_groups=[CORE_IDS],
                ins=[wu_in[:].opt()], outs=[wu_out[:].opt()])

            # ================= pooling =================
            nt_dr = dram.tile([POS, 64], F32)
            with tc.tile_pool(name="poolstage", bufs=1) as pp:
                x_t = pp.tile([64, 4096], F32, tag="pX")
                nc.scalar.dma_start(x_t[:], xc_d[:])
                # column (y) max-tree: window 7 rows
                s1 = pp.tile([64, 4032], F32, tag="pA")
                nc.vector.tensor_tensor(s1[:], x_t[:, 0:4032], x_t[:, 64:4096],
                                        op=ALU.max)
                s2 = pp.tile([64, 3904], F32, tag="pX")
                nc.vector.tensor_tensor(s2[:], s1[:, 0:3904], s1[:, 128:4032],
                                        op=ALU.max)
                s3 = pp.tile([64, POS], F32, tag="pA")
                nc.vector.tensor_tensor(s3[:], s2[:, 0:POS], s2[:, 192:3904],
                                        op=ALU.max)
                # row (x) max-tree: window 7 cols
                t1 = pp.tile([64, 3711], F32, tag="pX")
                nc.vector.tensor_tensor(t1[:], s3[:, 0:3711], s3[:, 1:3712],
                                        op=ALU.max)
                t2 = pp.tile([64, 3709], F32, tag="pA")
                nc.vector.tensor_tensor(t2[:], t1[:, 0:3709], t1[:, 2:3711],
                                        op=ALU.max)
                n_t = pp.tile([64, POS], F32, tag="pX")
                nc.vector.tensor_tensor(n_t[:, 0:3706], t2[:, 0:3706],
                                        t2[:, 3:3709], op=ALU.max)

                # ===== NT (transpose of pooled map) =====
                nt_sb = pp.tile([128, 29 * 64], F32)
                for q in range(29):
                    tr_ps = ps.tile([128, 64], F32, name=f"ntp{q}", tag="mm")
                    nc.tensor.transpose(tr_ps[:], n_t[:, 128 * q:128 * q + 128],
                                        id_t[0:64, 0:64])
                    if q % 2 == 0:
                        nc.vector.tensor_copy(nt_sb[:, 64 * q:64 * q + 64],
                                              tr_ps[:])
                    else:
                        nc.scalar.copy(nt_sb[:, 64 * q:64 * q + 64], tr_ps[:])
                nc.sync.dma_start(
                    nt_dr[:].rearrange("(q p) c -> p q c", p=128),
                    nt_sb[:].rearrange("p (q c) -> p q c", c=64))
            # w6 fold in a short-lived stage pool (space freed by poolstage)
            with tc.tile_pool(name="w6stage", bufs=1) as wsp:
                w6bs_t = wsp.tile([128, 16 * 512], F32)
                nc.scalar.dma_start(
                    w6bs_t[:].rearrange("p (q e) -> p q e", e=512),
                    w6b_d[:].rearrange("(q p) e -> p q e", p=128))
                nc.vector.tensor_add(w6e_t[:], w6e_t[:], w6bs_t[:])


            # ================= chunked pipeline =================
            agf_in = [dram.tile([F_LOC, RC // 2], F32, name=f"agfi{j}")
                      for j in range(2 * NCH)]
            agf_out = [dram.tile([2048, RC // 2], F32, addr_space="Shared",
                                 name=f"agfo{j}") for j in range(2 * NCH)]
            agh_in = [dram.tile([512, RC // 2], BF16, name=f"aghi{j}")
                      for j in range(2 * NCH)]
            agh_out = [dram.tile([4096, RC // 2], BF16, addr_space="Shared",
                                 name=f"agho{j}") for j in range(2 * NCH)]
            ar_in = [dram.tile([NH, RC], F32, name=f"ari{j}")
                     for j in range(NCH)]
            ar_out = [dram.tile([NH, RC], F32, addr_space="Shared",
                                name=f"aro{j}") for j in range(NCH)]

            xr_sb = sb.tile([NH, R], F32, tag="x21a")

            # ---- Phase A: gathers + transposes + AGf triggers (all quarters)
            for j in range(2 * NCH):
                ftT = [st.tile([128, RC // 2], F32, name=f"ftT{j}_{t}",
                               tag=f"ftT{t}", bufs=2) for t in range(2)]
                for bb in range(4):
                    b = 4 * j + bb
                    blk = st.tile([128, 256], F32, name=f"gblk{b}", tag="gblk",
                                  bufs=2)
                    for k in range(4):
                        nc.gpsimd.indirect_dma_start(
                            out=blk[:, 64 * k:64 * k + 64], out_offset=None,
                            in_=nt_dr[:],
                            in_offset=bass.IndirectOffsetOnAxis(
                                ap=idx_t[:, 16 * k + b:16 * k + b + 1], axis=0))
                    for t in range(2):
                        trp = ps.tile([128, 128], F32, name=f"ftp{b}_{t}",
                                      tag="mm")
                        nc.tensor.transpose(trp[:],
                                            blk[:, 128 * t:128 * t + 128],
                                            id_t[:])
                        if (2 * bb + t) % 2 == 0:
                            nc.vector.tensor_copy(
                                ftT[t][:, 128 * bb:128 * bb + 128], trp[:])
                        else:
                            nc.scalar.copy(ftT[t][:, 128 * bb:128 * bb + 128],
                                           trp[:])
                for t in range(2):
                    nc.scalar.dma_start(agf_in[j][128 * t:128 * t + 128, :],
                                        ftT[t][:])
                nc.gpsimd.collective_compute(
                    "AllGather", ALU.bypass, replica_groups=[CORE_IDS],
                    ins=[agf_in[j][:].opt()], outs=[agf_out[j][:].opt()])

            # w7 f32->bf16 cast-DMA after all gather descriptors are queued
            w7_t = wp.tile([128, 32 * 512], BF16)
            nc.gpsimd.dma_start(
                w7_t[:].rearrange("p (q e) -> p q e", e=512),
                w7s_d[:].rearrange("(q p) e -> p q e", p=128))

            # ---- Phase B: FC6 halves (fp32r) + per-half AGh triggers
            for jj in range(2 * NCH):
                p6 = [ps.tile([128, 512], F32, name=f"p6_{jj}_{i}", tag="mm")
                      for i in range(4)]
                for g in range(8):
                    ft = st.tile([128, 2 * 512], F32R, name=f"ft{jj}_{g}",
                                 tag="fc6ft", bufs=3)
                    nc.sync.dma_start(
                        ft[:].rearrange("p (q r) -> p q r", r=512),
                        agf_out[jj][256 * g:256 * g + 256, :].bitcast(
                            F32R).rearrange("(q p) r -> p q r", p=128))
                    for m in range(4):
                        for qq in range(2):
                            q = 2 * g + qq
                            nc.tensor.matmul(
                                p6[m][:],
                                w6e_t[:, 512 * q + 128 * m:
                                      512 * q + 128 * m + 128],
                                ft[:, 512 * qq:512 * qq + 512],
                                start=(q == 0), stop=(q == 15))
                h1h = st.tile([128, 4 * 512], BF16, name=f"h1h{jj}",
                              tag="h1h", bufs=2)
                for m in range(4):
                    nc.scalar.activation(
                        h1h[:, 512 * m:512 * m + 512],
                        p6[m][:], AF.Relu, b6_t[:, m:m + 1], 1.0)
                    nc.sync.dma_start(
                        agh_in[jj][128 * m:128 * m + 128, :],
                        h1h[:, 512 * m:512 * m + 512])
                nc.gpsimd.collective_compute(
                    "AllGather", ALU.bypass, replica_groups=[CORE_IDS],
                    ins=[agh_in[jj][:].opt()],
                    outs=[agh_out[jj][:].opt()])

            # ---- Phase C: FC7 (bf16) + FC8 (fp32r) + paired AR per 2 chunks
            for j in range(NCH):
                h2j = [st.tile([128, RC], F32R, name=f"h2j{j}_{m}",
                               tag=f"h2j{m}", bufs=1) for m in range(4)]
                for h in range(2):
                    p7 = [ps.tile([128, 512], F32, name=f"p7_{j}_{h}_{i}",
                                  tag="mm") for i in range(4)]
                    for g in range(8):
                        h1f = st.tile([128, 4 * 512], BF16,
                                      name=f"h1f{j}_{h}_{g}", tag="fc7h1",
                                      bufs=3)
                        nc.sync.dma_start(
                            h1f[:].rearrange("p (q r) -> p q r", r=512),
                            agh_out[2 * j + h][512 * g:512 * g + 512,
                                               :].rearrange(
                                "(q p) r -> p q r", p=128))
                        for m in range(4):
                            for qq in range(4):
                                q = 4 * g + qq
                                nc.tensor.matmul(
                                    p7[m][:],
                                    w7_t[:, 512 * q + 128 * m:
                                          512 * q + 128 * m + 128],
                                    h1f[:, 512 * qq:512 * qq + 512],
                                    start=(q == 0), stop=(q == 31))
                    for m in range(4):
                        nc.scalar.activation(
                            h2j[m][:, 512 * h:512 * h + 512],
                            p7[m][:], AF.Relu, b7_t[:, m:m + 1], 1.0)

                xaj = st.tile([NH, RC], F32, name=f"xaj{j}", tag="xaj", bufs=1)
                for n in range(2):
                    p8 = ps.tile([NH, 512], F32, name=f"p8_{j}_{n}", tag="mm")
                    for q in range(4):
                        nc.tensor.matmul(
                            p8[:], w8_t[:, NH * q:NH * q + NH],
                            h2j[q][:, 512 * n:512 * n + 512],
                            start=(q == 0), stop=(q == 3))
                    nc.scalar.copy(xaj[:, 512 * n:512 * n + 512], p8[:])
                nc.scalar.dma_start(ar_in[j][:], xaj[:])
                nc.gpsimd.collective_compute(
                    "AllReduce", ALU.add, replica_groups=[CORE_IDS],
                    ins=[ar_in[j][:].opt()], outs=[ar_out[j][:].opt()])
                nc.sync.dma_start(xr_sb[:, RC * j:RC * j + RC], ar_out[j][:])

            # ================= heads: bias + relu, ROI softmax stats =========
            xab = sb.tile([NH, R], F32, tag="xa42")
            nc.scalar.activation(xab[:], xr_sb[:], AF.Relu, b8_t[:], 1.0)
            xcT = xab[0:21, :]
            xdT = xab[32:53, :]
            md = sb.tile([21, 1], F32)
            nc.vector.tensor_reduce(md[:], xdT, axis=AX.X, op=ALU.max)
            nmd = sb.tile([21, 1], F32)
            nc.vector.tensor_scalar_mul(nmd[:], md[:], -1.0)
            exd = sb.tile([21, R], F32, tag="x21b")
            sd = sb.tile([21, 1], F32)
            nc.scalar.activation(exd[:], xdT, AF.Exp, nmd[:], 1.0,
                                 accum_out=sd[:])
            rsd = sb.tile([21, 1], F32)
            nc.vector.reciprocal(rsd[:], sd[:])
            smT = exd
            nc.vector.tensor_scalar_mul(smT[:], exd[:], rsd[:])

            # ================= class softmax via broadcast ops ==========
            ones_t = sb.tile([128, 1], F32)
            nc.vector.memset(ones_t[:], 1.0)
            xc_all = sb.tile([128, 16 * NCLS], F32)
            sm_all = sb.tile([128, 16 * NCLS], F32)
            dr_all = sb.tile([128, 16 * NCLS], F32)
            dm_all = sb.tile([128, 16 * NCLS], F32)
            sc_ps = ps.tile([NCLS, 1], F32, tag="mm")
            for b in range(16):
                pxc = ps.tile([128, NCLS], F32, name=f"pxc{b}", tag="mm")
                nc.tensor.transpose(pxc[:], xcT[:, 128 * b:128 * b + 128],
                                    id_t[0:21, 0:21])
                psm = ps.tile([128, NCLS], F32, name=f"psm{b}", tag="mm")
                nc.tensor.transpose(psm[:], smT[:, 128 * b:128 * b + 128],
                                    id_t[0:21, 0:21])
                if b % 2 == 0:
                    nc.vector.tensor_copy(xc_all[:, NCLS * b:NCLS * b + NCLS],
                                          pxc[:])
                    nc.scalar.copy(sm_all[:, NCLS * b:NCLS * b + NCLS], psm[:])
                else:
                    nc.scalar.copy(xc_all[:, NCLS * b:NCLS * b + NCLS], pxc[:])
                    nc.vector.tensor_copy(sm_all[:, NCLS * b:NCLS * b + NCLS],
                                          psm[:])
            xc3 = xc_all[:].rearrange("p (b j) -> p b j", j=NCLS)
            mxb = sb.tile([128, 16], F32)
            nc.vector.tensor_reduce(mxb[:], xc3, axis=AX.X, op=ALU.max)
            esub = sb.tile([128, 16 * NCLS], F32)
            nc.vector.tensor_tensor(
                esub[:].rearrange("p (b j) -> p b j", j=NCLS), xc3,
                mxb[:].rearrange("p (b o) -> p b o", o=1).to_broadcast([128, 16, NCLS]),
                op=ALU.subtract)
            nc.scalar.activation(esub[:], esub[:], AF.Exp, 0.0, 1.0)
            sxb = sb.tile([128, 16], F32)
            nc.vector.tensor_reduce(sxb[:],
                                    esub[:].rearrange("p (b j) -> p b j",
                                                      j=NCLS),
                                    axis=AX.X, op=ALU.add)
            rxb = sb.tile([128, 16], F32)
            nc.vector.reciprocal(rxb[:], sxb[:])
            nc.vector.tensor_tensor(
                dr_all[:].rearrange("p (b j) -> p b j", j=NCLS),
                esub[:].rearrange("p (b j) -> p b j", j=NCLS),
                rxb[:].rearrange("p (b o) -> p b o", o=1).to_broadcast([128, 16, NCLS]),
                op=ALU.mult)
            nc.vector.tensor_tensor(dm_all[:], dr_all[:], sm_all[:],
                                    op=ALU.mult)
            for b in range(16):
                nc.tensor.matmul(sc_ps[:],
                                 dm_all[:, NCLS * b:NCLS * b + NCLS],
                                 ones_t[:], start=(b == 0), stop=(b == 15))

            nc.sync.dma_start(
                dr_d[:].rearrange("(b p) j -> p b j", p=128),
                dr_all[:].rearrange("p (b j) -> p b j", j=NCLS))
            nc.sync.dma_start(
                dm_d[:].rearrange("(b p) j -> p b j", p=128),
                dm_all[:].rearrange("p (b j) -> p b j", j=NCLS))
            sc_sb = sb.tile([NCLS, 1], F32)
            nc.vector.tensor_copy(sc_sb[:], sc_ps[:])
            nc.sync.dma_start(score_d[:].rearrange("o j -> j o"), sc_sb[:])

    nc.compile()
    return nc


def _perm():
    f = np.arange(2048)
    # AG layout f' = 256*rank + 64*k + c  ->  original w6 input dim
    return 256 * (f // 256) + 4 * ((f % 256) % 64) + (f % 256) // 64


def kernel(x, w6, b6, w7, b7, w8c, b8c, w8d, b8d, ssw):
    global LAST_EXEC_NS
    if "nc" not in _CACHE:
        _CACHE["nc"] = _build()
    nc = _CACHE["nc"]

    x = np.asarray(x, dtype=np.float32)
    ssw_np = np.ascontiguousarray(np.asarray(ssw, dtype=np.int32)[0])
    w6T = np.asarray(w6, dtype=np.float32).T
    w7T = np.asarray(w7, dtype=np.float32).T
    w8T = np.zeros((4096, NH), dtype=np.float32)   # heads at cols 0:21, 32:53
    w8T[:, 0:21] = np.asarray(w8c, np.float32).T
    w8T[:, 32:53] = np.asarray(w8d, np.float32).T
    b6 = np.asarray(b6, np.float32)
    b7 = np.asarray(b7, np.float32)
    b8 = np.zeros(NH, dtype=np.float32)
    b8[0:21] = np.asarray(b8c, np.float32)
    b8[32:53] = np.asarray(b8d, np.float32)
    perm = _perm()
    ident = np.eye(128, dtype=np.float32)

    in_maps = []
    for e in CORE_IDS:
        cols = slice(512 * e, 512 * e + 512)
        in_maps.append({
            "xc": np.ascontiguousarray(
                x[0, 64 * e:64 * e + 64].reshape(C_LOC, 4096)),
            "ssw": ssw_np,
            "w6a": np.ascontiguousarray(w6T[perm, cols]),
            "w6b": np.ascontiguousarray(w6T[2048 + perm, cols]),
            "w7s": np.ascontiguousarray(w7T[:, cols]),
            "w8s": np.ascontiguousarray(w8T[512 * e:512 * e + 512, :]),
            "b6s": np.ascontiguousarray(b6[cols]),
            "b7s": np.ascontiguousarray(b7[cols]),
            "b8": b8,
            "ident": ident,
        })

    res = run_bass_kernel_spmd(nc, in_maps, CORE_IDS, trace=TRACE)
    LAST_EXEC_NS = res.exec_time_ns
    r0 = res.results[0]
    dm = r0["dm"].reshape(1, R, NCLS)
    dr = r0["dr"].reshape(1, R, NCLS)
    score = r0["score"].reshape(1, NCLS)
    return dm, dr, score


# revision 31
# speedup vs baseline: 1.0111x; 1.0111x over previous
"""SPP-net detection head (pooling + FC stack + dual softmax) on 8 TRN2 cores.

Strategy (tensor-parallel, all compute on device, ~560 us HW exec):
  - The per-ROI 7x7/s7 max-pool of a 14x14 crop == 4 point-lookups into a
    dense pooled map N[c, y*64+x] = max of the 7x7 window at (y, x), computed
    once via two 3-op max-trees (DVE). Channel-sharded: 64 channels/core,
    then PE-transposed to NT[pos, ch] in DRAM.
  - ROI gather: native indirect-DMA row-gathers straight from NT, one
    [128 ROIs, 64 ch] gather per (ROI-block, corner) with the corner offset
    folded into the device-computed index column (64 gathers total, no
    extended-ISA library, no table build).
  - Pipeline over 2 ROI chunks of 1024, software-pipelined emission so each
    engine FIFO overlaps: gathers+transposes+AGf triggers first, then
    FC6+AGh per chunk, then FC7+FC8+AR per chunk.
    FC6 fp32r (TP by output, folded w6eff = w6[:, :2048]+w6[:, 2048:] via
    DVE add), h1 wire + FC7 in bf16, FC8 fp32r with both heads fused into a
    padded [64, r] tile (partitions 0:21 / 32:53), one AllReduce per chunk.
  - A tiny dummy AllGather at kernel start absorbs the ncfw first-call cost.
  - Softmax-over-ROIs in the transposed [21, 2048] layout (free-dim
    reduction); PE-transpose to [r, 21]; class softmax via broadcast-AP
    vector ops; score via a ones-matmul accumulated in PSUM.

Host side does layout-only work: slicing, transposition, and a feature
permutation matching the corner-major gather order (f_local = k*64 + c).
"""
import os
import sys

import numpy as np

for _p in ("/opt/trn_rl_repo", "/root/.axon_site/_ro/trn_rl_repo"):
    if os.path.isdir(_p) and _p not in sys.path:
        sys.path.append(_p)

import concourse.bacc as bacc
import concourse.bass as bass
import concourse.mybir as mybir
import concourse.tile as tile
from concourse.bass_utils import run_bass_kernel_spmd

F32 = mybir.dt.float32
F32R = mybir.dt.float32r
BF16 = mybir.dt.bfloat16
I32 = mybir.dt.int32
AF = mybir.ActivationFunctionType
ALU = mybir.AluOpType
AX = mybir.AxisListType

NCORES = 8
CORE_IDS = list(range(NCORES))
R = 2048          # ROIs
NCH = 2           # ROI chunks in the pipeline
RC = R // NCH     # 512 ROIs per chunk
C_LOC = 64        # channels per core
F_LOC = 256       # features per core (4 corners x 64 ch)
NCLS = 21
NH = 64           # padded dual-head dim (xc at 0:21, xd at 32:53)
POS = 3712        # pooled-map positions (58 rows x 64 cols, x<=57 valid)
CKS = (0, 7, 448, 455)   # corner offsets: 7*64*(k//2) + 7*(k%2)

TRACE = False
LAST_EXEC_NS = None

_CACHE = {}


def _build():
    nc = bacc.Bacc("TRN2", target_bir_lowering=False, debug=False,
                   num_devices=NCORES)

    # ---- I/O ----
    xc_d = nc.dram_tensor("xc", [C_LOC, 4096], F32, kind="ExternalInput")
    ssw_d = nc.dram_tensor("ssw", [R, 4], I32, kind="ExternalInput")
    w6a_d = nc.dram_tensor("w6a", [2048, 512], F32, kind="ExternalInput")
    w6b_d = nc.dram_tensor("w6b", [2048, 512], F32, kind="ExternalInput")
    w7s_d = nc.dram_tensor("w7s", [4096, 512], F32, kind="ExternalInput")
    w8s_d = nc.dram_tensor("w8s", [512, NH], F32, kind="ExternalInput")
    b6s_d = nc.dram_tensor("b6s", [512], F32, kind="ExternalInput")
    b7s_d = nc.dram_tensor("b7s", [512], F32, kind="ExternalInput")
    b8_d = nc.dram_tensor("b8", [NH], F32, kind="ExternalInput")
    ident_d = nc.dram_tensor("ident", [128, 128], F32, kind="ExternalInput")

    dm_d = nc.dram_tensor("dm", [R, NCLS], F32, kind="ExternalOutput")
    dr_d = nc.dram_tensor("dr", [R, NCLS], F32, kind="ExternalOutput")
    score_d = nc.dram_tensor("score", [1, NCLS], F32, kind="ExternalOutput")

    with tile.TileContext(nc) as tc:
        with (
            tc.tile_pool(name="wp", bufs=1) as wp,        # persistent weights
            tc.tile_pool(name="sb", bufs=1) as sb,        # misc working tiles
            tc.tile_pool(name="st", bufs=2) as st,        # streaming tiles
            tc.tile_pool(name="ps", bufs=8, space="PSUM") as ps,
            tc.tile_pool(name="dram", bufs=1, space="DRAM") as dram,
        ):
            # ================= ROI indices + small loads ============
            sswf = ssw_d[:].rearrange("r c -> (r c)")
            y0_t = sb.tile([128, 16], I32)
            nc.sync.dma_start(y0_t[:], sswf.rearrange("(j p c) -> p j c",
                                                      p=128, c=4)[:, :, 0])
            x0_t = sb.tile([128, 16], I32)
            nc.sync.dma_start(x0_t[:], sswf.rearrange("(j p c) -> p j c",
                                                      p=128, c=4)[:, :, 1])
            id_t = wp.tile([128, 128], F32)
            nc.sync.dma_start(id_t[:], ident_d[:])
            y0f = sb.tile([128, 16], F32)
            nc.vector.tensor_copy(y0f[:], y0_t[:])
            x0f = sb.tile([128, 16], F32)
            nc.vector.tensor_copy(x0f[:], x0_t[:])
            basef = sb.tile([128, 16], F32)
            nc.vector.scalar_tensor_tensor(basef[:], y0f[:], 64.0, x0f[:],
                                           op0=ALU.mult, op1=ALU.add)
            idx4f = sb.tile([128, 64], F32)
            for k, ck in enumerate(CKS):
                nc.vector.tensor_scalar_add(idx4f[:, 16 * k:16 * k + 16],
                                            basef[:], float(ck))
            idx_t = sb.tile([128, 64], I32)
            nc.vector.tensor_copy(idx_t[:], idx4f[:])

            # x on the scalar HWDGE queue so pooling starts immediately;
            # big weight loads go on sync behind the tiny index loads.
            w6e_t = wp.tile([128, 16 * 512], F32R)
            nc.sync.dma_start(
                w6e_t[:].rearrange("p (q e) -> p q e", e=512),
                w6a_d[:].bitcast(F32R).rearrange("(q p) e -> p q e", p=128))
            w8_t = wp.tile([128, 4 * NH], F32R)
            nc.scalar.dma_start(
                w8_t[:].rearrange("p (q c) -> p q c", c=NH),
                w8s_d[:].bitcast(F32R).rearrange("(q p) c -> p q c", p=128))
            b6_t = wp.tile([128, 4], F32)
            nc.scalar.dma_start(b6_t[:], b6s_d[:].rearrange("(m p) -> p m", p=128))
            b7_t = wp.tile([128, 4], F32)
            nc.scalar.dma_start(b7_t[:], b7s_d[:].rearrange("(m p) -> p m", p=128))
            b8_t = wp.tile([NH, 1], F32)
            nc.scalar.dma_start(b8_t[:], b8_d[:].rearrange("(p o) -> p o", o=1))

            # tiny dummy collective to absorb ncfw first-call warmup early
            wu_in = dram.tile([16, 16], F32)
            wu_out = dram.tile([128, 16], F32, addr_space="Shared")
            nc.sync.dma_start(wu_in[:], id_t[0:16, 0:16])
            nc.gpsimd.collective_compute(
                "AllGather", ALU.bypass, replica_groups=[CORE_IDS],
                ins=[wu_in[:].opt()], outs=[wu_out[:].opt()])

            # ================= pooling =================
            nt_dr = dram.tile([POS, 64], F32)
            with tc.tile_pool(name="poolstage", bufs=1) as pp:
                x_t = pp.tile([64, 4096], F32, tag="pX")
                nc.scalar.dma_start(x_t[:], xc_d[:])
                # column (y) max-tree: window 7 rows
                s1 = pp.tile([64, 4032], F32, tag="pA")
                nc.vector.tensor_tensor(s1[:], x_t[:, 0:4032], x_t[:, 64:4096],
                                        op=ALU.max)
                s2 = pp.tile([64, 3904], F32, tag="pX")
                nc.vector.tensor_tensor(s2[:], s1[:, 0:3904], s1[:, 128:4032],
                                        op=ALU.max)
                s3 = pp.tile([64, POS], F32, tag="pA")
                nc.vector.tensor_tensor(s3[:], s2[:, 0:POS], s2[:, 192:3904],
                                        op=ALU.max)
                # row (x) max-tree: window 7 cols
                t1 = pp.tile([64, 3711], F32, tag="pX")
                nc.vector.tensor_tensor(t1[:], s3[:, 0:3711], s3[:, 1:3712],
                                        op=ALU.max)
                t2 = pp.tile([64, 3709], F32, tag="pA")
                nc.vector.tensor_tensor(t2[:], t1[:, 0:3709], t1[:, 2:3711],
                                        op=ALU.max)
                n_t = pp.tile([64, POS], F32, tag="pX")
                nc.vector.tensor_tensor(n_t[:, 0:3706], t2[:, 0:3706],
                                        t2[:, 3:3709], op=ALU.max)

                # ===== NT (transpose of pooled map) =====
                nt_sb = pp.tile([128, 29 * 64], F32)
                for q in range(29):
                    tr_ps = ps.tile([128, 64], F32, name=f"ntp{q}", tag="mm")
                    nc.tensor.transpose(tr_ps[:], n_t[:, 128 * q:128 * q + 128],
                                        id_t[0:64, 0:64])
                    if q % 2 == 0:
                        nc.vector.tensor_copy(nt_sb[:, 64 * q:64 * q + 64],
                                              tr_ps[:])
                    else:
                        nc.scalar.copy(nt_sb[:, 64 * q:64 * q + 64], tr_ps[:])
                nc.sync.dma_start(
                    nt_dr[:].rearrange("(q p) c -> p q c", p=128),
                    nt_sb[:].rearrange("p (q c) -> p q c", c=64))
            # w6 fold in a short-lived stage pool (space freed by poolstage)
            with tc.tile_pool(name="w6stage", bufs=1) as wsp:
                w6bs_t = wsp.tile([128, 16 * 512], F32)
                nc.scalar.dma_start(
                    w6bs_t[:].rearrange("p (q e) -> p q e", e=512),
                    w6b_d[:].rearrange("(q p) e -> p q e", p=128))
                nc.vector.tensor_add(w6e_t[:], w6e_t[:], w6bs_t[:])


            # ================= chunked pipeline =================
            agf_in = [dram.tile([F_LOC, RC // 2], F32, name=f"agfi{j}")
                      for j in range(2 * NCH)]
            agf_out = [dram.tile([2048, RC // 2], F32, addr_space="Shared",
                                 name=f"agfo{j}") for j in range(2 * NCH)]
            agh_in = [dram.tile([512, RC // 2], BF16, name=f"aghi{j}")
                      for j in range(2 * NCH)]
            agh_out = [dram.tile([4096, RC // 2], BF16, addr_space="Shared",
                                 name=f"agho{j}") for j in range(2 * NCH)]
            ar_in = [dram.tile([NH, RC], F32, name=f"ari{j}")
                     for j in range(NCH)]
            ar_out = [dram.tile([NH, RC], F32, addr_space="Shared",
                                name=f"aro{j}") for j in range(NCH)]

            xr_sb = sb.tile([NH, R], F32, tag="x21a")

            # ---- Phase A: gathers + transposes + AGf triggers (all quarters)
            for j in range(2 * NCH):
                ftT = [st.tile([128, RC // 2], F32, name=f"ftT{j}_{t}",
                               tag=f"ftT{t}", bufs=2) for t in range(2)]
                for bb in range(4):
                    b = 4 * j + bb
                    blk = st.tile([128, 256], F32, name=f"gblk{b}", tag="gblk",
                                  bufs=2)
                    for k in range(4):
                        nc.gpsimd.indirect_dma_start(
                            out=blk[:, 64 * k:64 * k + 64], out_offset=None,
                            in_=nt_dr[:],
                            in_offset=bass.IndirectOffsetOnAxis(
                                ap=idx_t[:, 16 * k + b:16 * k + b + 1], axis=0))
                    for t in range(2):
                        trp = ps.tile([128, 128], F32, name=f"ftp{b}_{t}",
                                      tag="mm")
                        nc.tensor.transpose(trp[:],
                                            blk[:, 128 * t:128 * t + 128],
                                            id_t[:])
                        if (2 * bb + t) % 2 == 0:
                            nc.vector.tensor_copy(
                                ftT[t][:, 128 * bb:128 * bb + 128], trp[:])
                        else:
                            nc.scalar.copy(ftT[t][:, 128 * bb:128 * bb + 128],
                                           trp[:])
                for t in range(2):
                    nc.scalar.dma_start(agf_in[j][128 * t:128 * t + 128, :],
                                        ftT[t][:])
                nc.gpsimd.collective_compute(
                    "AllGather", ALU.bypass, replica_groups=[CORE_IDS],
                    ins=[agf_in[j][:].opt()], outs=[agf_out[j][:].opt()])

            # w7 f32->bf16 cast-DMA after all gather descriptors are queued
            w7_t = wp.tile([128, 32 * 512], BF16)
            nc.gpsimd.dma_start(
                w7_t[:].rearrange("p (q e) -> p q e", e=512),
                w7s_d[:].rearrange("(q p) e -> p q e", p=128))

            # ---- Phase B: FC6 halves (fp32r) + per-half AGh triggers
            for jj in range(2 * NCH):
                p6 = [ps.tile([128, 512], F32, name=f"p6_{jj}_{i}", tag="mm")
                      for i in range(4)]
                for g in range(8):
                    ft = st.tile([128, 2 * 512], F32R, name=f"ft{jj}_{g}",
                                 tag="fc6ft", bufs=3)
                    nc.sync.dma_start(
                        ft[:].rearrange("p (q r) -> p q r", r=512),
                        agf_out[jj][256 * g:256 * g + 256, :].bitcast(
                            F32R).rearrange("(q p) r -> p q r", p=128))
                    for m in range(4):
                        for qq in range(2):
                            q = 2 * g + qq
                            nc.tensor.matmul(
                                p6[m][:],
                                w6e_t[:, 512 * q + 128 * m:
                                      512 * q + 128 * m + 128],
                                ft[:, 512 * qq:512 * qq + 512],
                                start=(q == 0), stop=(q == 15))
                h1h = st.tile([128, 4 * 512], BF16, name=f"h1h{jj}",
                              tag="h1h", bufs=2)
                for m in range(4):
                    nc.scalar.activation(
                        h1h[:, 512 * m:512 * m + 512],
                        p6[m][:], AF.Relu, b6_t[:, m:m + 1], 1.0)
                    nc.sync.dma_start(
                        agh_in[jj][128 * m:128 * m + 128, :],
                        h1h[:, 512 * m:512 * m + 512])
                nc.gpsimd.collective_compute(
                    "AllGather", ALU.bypass, replica_groups=[CORE_IDS],
                    ins=[agh_in[jj][:].opt()],
                    outs=[agh_out[jj][:].opt()])

            # ---- Phase C: FC7 (bf16) + FC8 (fp32r) + paired AR per 2 chunks
            for j in range(NCH):
                h2j = [st.tile([128, RC], F32R, name=f"h2j{j}_{m}",
                               tag=f"h2j{m}", bufs=1) for m in range(4)]
                for h in range(2):
                    p7 = [ps.tile([128, 512], F32, name=f"p7_{j}_{h}_{i}",
                                  tag="mm") for i in range(4)]
                    for gp in range(4):
                        hf = []
                        for gg in range(2):
                            g = 2 * gp + gg
                            h1f = st.tile([128, 4 * 512], BF16,
                                          name=f"h1f{j}_{h}_{g}", tag="fc7h1",
                                          bufs=3)
                            nc.sync.dma_start(
                                h1f[:].rearrange("p (q r) -> p q r", r=512),
                                agh_out[2 * j + h][512 * g:512 * g + 512,
                                                   :].rearrange(
                                    "(q p) r -> p q r", p=128))
                            hf.append(h1f)
                        for m in range(4):
                            for qq8 in range(8):
                                q = 8 * gp + qq8
                                nc.tensor.matmul(
                                    p7[m][:],
                                    w7_t[:, 512 * q + 128 * m:
                                          512 * q + 128 * m + 128],
                                    hf[qq8 // 4][:, 512 * (qq8 % 4):
                                                 512 * (qq8 % 4) + 512],
                                    start=(q == 0), stop=(q == 31))
                    for m in range(4):
                        nc.scalar.activation(
                            h2j[m][:, 512 * h:512 * h + 512],
                            p7[m][:], AF.Relu, b7_t[:, m:m + 1], 1.0)

                xaj = st.tile([NH, RC], F32, name=f"xaj{j}", tag="xaj", bufs=1)
                for n in range(2):
                    p8 = ps.tile([NH, 512], F32, name=f"p8_{j}_{n}", tag="mm")
                    for q in range(4):
                        nc.tensor.matmul(
                            p8[:], w8_t[:, NH * q:NH * q + NH],
                            h2j[q][:, 512 * n:512 * n + 512],
                            start=(q == 0), stop=(q == 3))
                    nc.scalar.copy(xaj[:, 512 * n:512 * n + 512], p8[:])
                nc.scalar.dma_start(ar_in[j][:], xaj[:])
                nc.gpsimd.collective_compute(
                    "AllReduce", ALU.add, replica_groups=[CORE_IDS],
                    ins=[ar_in[j][:].opt()], outs=[ar_out[j][:].opt()])
                nc.sync.dma_start(xr_sb[:, RC * j:RC * j + RC], ar_out[j][:])

            # ================= heads: bias + relu, ROI softmax stats =========
            xab = sb.tile([NH, R], F32, tag="xa42")
            nc.scalar.activation(xab[:], xr_sb[:], AF.Relu, b8_t[:], 1.0)
            xcT = xab[0:21, :]
            xdT = xab[32:53, :]
            md = sb.tile([21, 1], F32)
            nc.vector.tensor_reduce(md[:], xdT, axis=AX.X, op=ALU.max)
            nmd = sb.tile([21, 1], F32)
            nc.vector.tensor_scalar_mul(nmd[:], md[:], -1.0)
            exd = sb.tile([21, R], F32, tag="x21b")
            sd = sb.tile([21, 1], F32)
            nc.scalar.activation(exd[:], xdT, AF.Exp, nmd[:], 1.0,
                                 accum_out=sd[:])
            rsd = sb.tile([21, 1], F32)
            nc.vector.reciprocal(rsd[:], sd[:])
            smT = exd
            nc.vector.tensor_scalar_mul(smT[:], exd[:], rsd[:])

            # ================= class softmax via broadcast ops ==========
            ones_t = sb.tile([128, 1], F32)
            nc.vector.memset(ones_t[:], 1.0)
            xc_all = sb.tile([128, 16 * NCLS], F32)
            sm_all = sb.tile([128, 16 * NCLS], F32)
            dr_all = sb.tile([128, 16 * NCLS], F32)
            dm_all = sb.tile([128, 16 * NCLS], F32)
            sc_ps = ps.tile([NCLS, 1], F32, tag="mm")
            for b in range(16):
                pxc = ps.tile([128, NCLS], F32, name=f"pxc{b}", tag="mm")
                nc.tensor.transpose(pxc[:], xcT[:, 128 * b:128 * b + 128],
                                    id_t[0:21, 0:21])
                psm = ps.tile([128, NCLS], F32, name=f"psm{b}", tag="mm")
                nc.tensor.transpose(psm[:], smT[:, 128 * b:128 * b + 128],
                                    id_t[0:21, 0:21])
                if b % 2 == 0:
                    nc.vector.tensor_copy(xc_all[:, NCLS * b:NCLS * b + NCLS],
                                          pxc[:])
                    nc.scalar.copy(sm_all[:, NCLS * b:NCLS * b + NCLS], psm[:])
                else:
                    nc.scalar.copy(xc_all[:, NCLS * b:NCLS * b + NCLS], pxc[:])
                    nc.vector.tensor_copy(sm_all[:, NCLS * b:NCLS * b + NCLS],
                                          psm[:])
            xc3 = xc_all[:].rearrange("p (b j) -> p b j", j=NCLS)
            mxb = sb.tile([128, 16], F32)
            nc.vector.tensor_reduce(mxb[:], xc3, axis=AX.X, op=ALU.max)
            esub = sb.tile([128, 16 * NCLS], F32)
            nc.vector.tensor_tensor(
                esub[:].rearrange("p (b j) -> p b j", j=NCLS), xc3,
                mxb[:].rearrange("p (b o) -> p b o", o=1).to_broadcast([128, 16, NCLS]),
                op=ALU.subtract)
            nc.scalar.activation(esub[:], esub[:], AF.Exp, 0.0, 1.0)
            sxb = sb.tile([128, 16], F32)
            nc.vector.tensor_reduce(sxb[:],
                                    esub[:].rearrange("p (b j) -> p b j",
                                                      j=NCLS),
                                    axis=AX.X, op=ALU.add)
            rxb = sb.tile([128, 16], F32)
            nc.vector.reciprocal(rxb[:], sxb[:])
            nc.vector.tensor_tensor(
                dr_all[:].rearrange("p (b j) -> p b j", j=NCLS),
                esub[:].rearrange("p (b j) -> p b j", j=NCLS),
                rxb[:].rearrange("p (b o) -> p b o", o=1).to_broadcast([128, 16, NCLS]),
                op=ALU.mult)
            nc.vector.tensor_tensor(dm_all[:], dr_all[:], sm_all[:],
                                    op=ALU.mult)
            for b in range(16):
                nc.tensor.matmul(sc_ps[:],
                                 dm_all[:, NCLS * b:NCLS * b + NCLS],
                                 ones_t[:], start=(b == 0), stop=(b == 15))

            nc.sync.dma_start(
                dr_d[:].rearrange("(b p) j -> p b j", p=128),
                dr_all[:].rearrange("p (b j) -> p b j", j=NCLS))
            nc.sync.dma_start(
                dm_d[:].rearrange("(b p) j -> p b j", p=128),
                dm_all[:].rearrange("p (b j) -> p b j", j=NCLS))
            sc_sb = sb.tile([NCLS, 1], F32)
            nc.vector.tensor_copy(sc_sb[:], sc_ps[:])
            nc.sync.dma_start(score_d[:].rearrange("o j -> j o"), sc_sb[:])

    nc.compile()
    return nc


def _perm():
    f = np.arange(2048)
    # AG layout f' = 256*rank + 64*k + c  ->  original w6 input dim
    return 256 * (f // 256) + 4 * ((f % 256) % 64) + (f % 256) // 64


def kernel(x, w6, b6, w7, b7, w8c, b8c, w8d, b8d, ssw):
    global LAST_EXEC_NS
    if "nc" not in _CACHE:
        _CACHE["nc"] = _build()
    nc = _CACHE["nc"]

    x = np.asarray(x, dtype=np.float32)
    ssw_np = np.ascontiguousarray(np.asarray(ssw, dtype=np.int32)[0])
    w6T = np.asarray(w6, dtype=np.float32).T
    w7T = np.asarray(w7, dtype=np.float32).T
    w8T = np.zeros((4096, NH), dtype=np.float32)   # heads at cols 0:21, 32:53
    w8T[:, 0:21] = np.asarray(w8c, np.float32).T
    w8T[:, 32:53] = np.asarray(w8d, np.float32).T
    b6 = np.asarray(b6, np.float32)
    b7 = np.asarray(b7, np.float32)
    b8 = np.zeros(NH, dtype=np.float32)
    b8[0:21] = np.asarray(b8c, np.float32)
    b8[32:53] = np.asarray(b8d, np.float32)
    perm = _perm()
    ident = np.eye(128, dtype=np.float32)

    in_maps = []
    for e in CORE_IDS:
        cols = slice(512 * e, 512 * e + 512)
        in_maps.append({
            "xc": np.ascontiguousarray(
                x[0, 64 * e:64 * e + 64].reshape(C_LOC, 4096)),
            "ssw": ssw_np,
            "w6a": np.ascontiguousarray(w6T[perm, cols]),
            "w6b": np.ascontiguousarray(w6T[2048 + perm, cols]),
            "w7s": np.ascontiguousarray(w7T[:, cols]),
            "w8s": np.ascontiguousarray(w8T[512 * e:512 * e + 512, :]),
            "b6s": np.ascontiguousarray(b6[cols]),
            "b7s": np.ascontiguousarray(b7[cols]),
            "b8": b8,
            "ident": ident,
        })

    res = run_bass_kernel_spmd(nc, in_maps, CORE_IDS, trace=TRACE)
    LAST_EXEC_NS = res.exec_time_ns
    r0 = res.results[0]
    dm = r0["dm"].reshape(1, R, NCLS)
    dr = r0["dr"].reshape(1, R, NCLS)
    score = r0["score"].reshape(1, NCLS)
    return dm, dr, score
